# revision 31
# baseline (speedup 1.0000x reference)
"""AllAtomFAPE loss kernel for Trainium2 (8 NeuronCores, SPMD).

Problem: b=1, N=384 res, F=8 frames/res -> NF=3072 frames; A=14 atoms/res
-> NA=5376 atoms. Output: scalar (shape (1,)) masked clamped FAPE.

Algorithm (factorized pairwise distance):
  With P = pR pR^T, T = tR tR^T, M = pR tR^T (per frame, 3x3),
    d2(f,a) = (pp-pt)^T P (pp-pt) + (tp-tt)^T T (tp-tt)
              - 2 (pp-pt)^T M (tp-tt)
  expands into a K=34 dot product between a frame feature vector W[:,f]
  and an atom feature vector Z[:,a]:
    rows 0-8   : P[i,j]            <->  pp_i pp_j
    rows 9-17  : T[i,j]            <->  tp_i tp_j
    rows 18-26 : M[i,j]            <->  -2 pp_i tp_j
    rows 27-29 : 2(M tt - P pt)    <->  pp
    rows 30-32 : 2(M^T pt - T tt)  <->  tp
    row  33    : c_f               <->  1
  so the whole pairwise computation is one (34 x NF) x (34 x NA) matmul
  on the TensorEngine (bf16). Then (ScalarE) d = m_a*sqrt(d2+eps) via
  sqrt(scale*x+bias) with per-partition scale=m^2, bias=m^2*eps, and
  (VectorE) fused clamp+reduce: tensor_scalar(min thr=10*m_a, accum add).

Sharding: atoms sharded across the 8 cores (672 each, padded to 768);
frames replicated. Each core emits one partial scalar; the host sums
the 8 partials (the gather/unshard step).

Layouts: frame f = 24*p + t (partition p, block t); slabs are
row-major in the feature index r with the block index t innermost
(unit stride) so DVE ops hit the packed 2x bf16 mode. Feature slabs
are transposed to [34, entity] via PE transposes (3 blocks packed per
transpose -> [102, 128] in PSUM) + strided DVE copies.
"""

import numpy as np

import concourse.bacc as bacc
import concourse.bass as bass
import concourse.tile as tile
from concourse import mybir
from concourse.bass_utils import run_bass_kernel_spmd

F32 = mybir.dt.float32
BF16 = mybir.dt.bfloat16
AX = mybir.AxisListType
OP = mybir.AluOpType
ACTF = mybir.ActivationFunctionType

NCORES = 8
NF = 3072          # frames (N*F)
TFB = 24           # frame blocks per partition (f = 24*p + t)
NA = 5376          # atoms (N*A)
NAS = NA // NCORES  # 672 atoms per core
NAPAD = 768        # padded per-core atoms
TAB = 6            # atom blocks per partition (a = 6*p + t)
K = 34             # feature dim
KS = 64            # stored feature rows (zero-padded, 128/KS-aligned for
                   # the DMA xbar transpose + 32-aligned strip copies)
CH = 1536          # frame chunk (PSUM cols) per ACT/DVE op
NCH = NF // CH     # 2 chunks
MMN = 512          # matmul moving free dim
X_FUSED = 10       # chunks (of 12) whose clamp+reduce is fused on DVE at 1x
                   # (measured cheapest reduction: ACT Copy+accum ~2.4us/chunk
                   # and PE ones-matmuls both regressed).
EPS = 1e-4
EPS_EFF = EPS      # sqrt(neg)=NaN is filtered by the DVE min (min(NaN,thr)=thr,
                   # verified on HW); rare tiny-d2 pairs hitting that path
                   # contribute ~1e-4 relative error.
CLAMP = 10.0
ZSCALE = 10.0
CNORM = float(1.0 / (ZSCALE * (3072.0 + EPS)))


def _bc(ap, dim, n):
    """Broadcast AP along a new axis at position `dim` (stride-0), n copies."""
    return ap.unsqueeze(dim).to_broadcast(
        tuple(ap.shape[:dim]) + (n,) + tuple(ap.shape[dim:])
    )


def build_nc():
    nc = bacc.Bacc(None)

    pr_d = nc.declare_dram_parameter("pr", [128, 9 * TFB], F32, isOutput=False)
    tr_d = nc.declare_dram_parameter("tr", [128, 9 * TFB], F32, isOutput=False)
    pt_d = nc.declare_dram_parameter("pt", [128, 3 * TFB], F32, isOutput=False)
    tt_d = nc.declare_dram_parameter("tt", [128, 3 * TFB], F32, isOutput=False)
    pp_d = nc.declare_dram_parameter("pp", [128, 3 * TAB], F32, isOutput=False)
    tp_d = nc.declare_dram_parameter("tp", [128, 3 * TAB], F32, isOutput=False)
    am_d = nc.declare_dram_parameter("am", [128, TAB], F32, isOutput=False)
    amf_d = nc.declare_dram_parameter("amf", [128, NA // 128], F32, isOutput=False)
    id_d = nc.declare_dram_parameter("ident", [128, 128], F32, isOutput=False)
    out_d = nc.declare_dram_parameter("out", [1, 2], F32, isOutput=True)

    with tile.TileContext(nc) as tc:
        with (
            tc.tile_pool(name="consts", bufs=1) as consts,
            tc.tile_pool(name="feat", bufs=1) as feat,
            tc.tile_pool(name="psum", bufs=2, space="PSUM") as psum_pool,
            tc.tile_pool(name="sbuf_s", bufs=3) as sbuf_s,
        ):
            # ---------------- input DMAs (split across queues) ----------
            pRs = consts.tile([128, 9 * TFB], F32)
            tRs = consts.tile([128, 9 * TFB], F32)
            pts = consts.tile([128, 3 * TFB], F32)
            tts = consts.tile([128, 3 * TFB], F32)
            pps = consts.tile([128, 3 * TAB], F32)
            tps = consts.tile([128, 3 * TAB], F32)
            ams = consts.tile([128, TAB], F32)
            amf = consts.tile([128, NA // 128], F32)
            nc.sync.dma_start(out=pRs[:], in_=pr_d[:])
            nc.scalar.dma_start(out=tRs[:], in_=tr_d[:])
            nc.sync.dma_start(out=pts[:], in_=pt_d[:])
            nc.scalar.dma_start(out=tts[:], in_=tt_d[:])
            nc.sync.dma_start(out=pps[:], in_=pp_d[:])
            nc.scalar.dma_start(out=tps[:], in_=tp_d[:])
            nc.sync.dma_start(out=ams[:], in_=am_d[:])
            nc.scalar.dma_start(out=amf[:], in_=amf_d[:])

            identf = consts.tile([128, 128], F32)
            nc.scalar.dma_start(out=identf[:], in_=id_d[:])
            identity = consts.tile([128, 128], BF16)
            nc.vector.tensor_copy(identity[:], identf[:])
            pRb, tRb, ptb, ttb, ppb, tpb = pRs, tRs, pts, tts, pps, tps

            # ------------- frame features (fp32, two t-halves) ----------
            # Slab col = KS*t + r. Computed per t-half so the first half's
            # transposes + main-loop chunk ch=0 overlap the second half.
            Wslab = feat.tile([128, KS * TFB], F32)
            nc.vector.memset(Wslab[:], 0.0)
            TH = TFB // 2
            mul0 = feat.tile([128, 9 * TH], F32)
            mul1 = feat.tile([128, 9 * TH], F32)
            mul2 = feat.tile([128, 9 * TH], F32)
            tmp3b = feat.tile([128, 3 * TH], F32)
            tmp3 = feat.tile([128, 3 * TFB], F32)
            tmp1 = feat.tile([128, TFB], F32)
            Ppt = feat.tile([128, 3 * TFB], F32)
            Mtt = feat.tile([128, 3 * TFB], F32)
            Ttt = feat.tile([128, 3 * TFB], F32)
            Mtp = feat.tile([128, 3 * TFB], F32)

            W4a = Wslab[:].rearrange("p (t r) -> p r t", r=KS)         # [128,64,24]
            R4a = pRb[:].rearrange("p (c t) -> p c t", c=9)
            T4a = tRb[:].rearrange("p (c t) -> p c t", c=9)
            pt3a = ptb[:].rearrange("p (c t) -> p c t", c=3)           # [128,3,24]
            tt3a = ttb[:].rearrange("p (c t) -> p c t", c=3)
            m0v = mul0[:].rearrange("p (i j t) -> p i j t", i=3, j=3)  # contig
            m1v = mul1[:].rearrange("p (i j t) -> p i j t", i=3, j=3)
            m2v = mul2[:].rearrange("p (i j t) -> p i j t", i=3, j=3)
            t3bv = tmp3b[:].rearrange("p (c t) -> p c t", c=3)

            def frame_feats(lo, hi):
                W4 = W4a[:, :, lo:hi]
                R4 = R4a[:, :, lo:hi].rearrange("p (i k) t -> p i k t", i=3)
                T4 = T4a[:, :, lo:hi].rearrange("p (i k) t -> p i k t", i=3)
                pt3 = pt3a[:, :, lo:hi]
                tt3 = tt3a[:, :, lo:hi]

                def gram(out4, A4, B4):
                    # 3 muls to contiguous temps, adds; only the last add
                    # writes the (KS-strided) slab rows.
                    a = lambda k: _bc(A4[:, :, k, :], 2, 3)
                    b = lambda k: _bc(B4[:, :, k, :], 1, 3)
                    nc.vector.tensor_mul(m0v, a(0), b(0))
                    nc.vector.tensor_mul(m1v, a(1), b(1))
                    nc.vector.tensor_mul(m2v, a(2), b(2))
                    nc.vector.tensor_add(m0v, m0v, m1v)
                    nc.vector.tensor_add(out4, m0v, m2v)

                Pv = W4[:, 0:9, :].rearrange("p (i j) t -> p i j t", i=3)
                Tv = W4[:, 9:18, :].rearrange("p (i j) t -> p i j t", i=3)
                Mv = W4[:, 18:27, :].rearrange("p (i j) t -> p i j t", i=3)
                gram(Pv, R4, R4)
                gram(Tv, T4, T4)
                gram(Mv, R4, T4)

                def matvec(out3, Q, vec3, transpose=False):
                    q = (lambda j: Q[:, :, j, :]) if not transpose else (lambda j: Q[:, j, :, :])
                    v = lambda j: _bc(vec3[:, j, :], 1, 3)
                    nc.vector.tensor_mul(out3, q(0), v(0))
                    for j in (1, 2):
                        nc.vector.tensor_mul(t3bv, q(j), v(j))
                        nc.vector.tensor_add(out3, out3, t3bv)

                Ppt3 = Ppt[:].rearrange("p (c t) -> p c t", c=3)[:, :, lo:hi]
                Mtt3 = Mtt[:].rearrange("p (c t) -> p c t", c=3)[:, :, lo:hi]
                Ttt3 = Ttt[:].rearrange("p (c t) -> p c t", c=3)[:, :, lo:hi]
                Mtp3 = Mtp[:].rearrange("p (c t) -> p c t", c=3)[:, :, lo:hi]
                matvec(Ppt3, Pv, pt3)
                matvec(Mtt3, Mv, tt3)
                matvec(Ttt3, Tv, tt3)
                matvec(Mtp3, Mv, pt3, transpose=True)  # M^T pt

                tmp3v = tmp3[:].rearrange("p (c t) -> p c t", c=3)[:, :, lo:hi]
                nc.vector.tensor_sub(tmp3v, Mtt3, Ppt3)
                nc.vector.tensor_scalar_mul(W4[:, 27:30, :], tmp3v, 2.0)
                nc.vector.tensor_sub(tmp3v, Mtp3, Ttt3)
                nc.vector.tensor_scalar_mul(W4[:, 30:33, :], tmp3v, 2.0)

                # cf row 33: pt.(Ppt - 2*Mtt) + tt.Ttt
                cfb = W4[:, 33, :]
                t1b = tmp1[:, lo:hi]
                nc.vector.tensor_sub(tmp3v, Ppt3, Mtt3)
                nc.vector.tensor_sub(tmp3v, tmp3v, Mtt3)
                nc.vector.tensor_mul(cfb, tmp3v[:, 0, :], pt3[:, 0, :])
                for c in (1, 2):
                    nc.vector.tensor_mul(t1b, tmp3v[:, c, :], pt3[:, c, :])
                    nc.vector.tensor_add(cfb, cfb, t1b)
                for c in (0, 1, 2):
                    nc.vector.tensor_mul(t1b, Ttt3[:, c, :], tt3[:, c, :])
                    nc.vector.tensor_add(cfb, cfb, t1b)

            # ------------- atom features (sharded) ----------------------
            Zslab = feat.tile([128, KS * TAB], F32)
            nc.vector.memset(Zslab[:], 0.0)
            Z4 = Zslab[:].rearrange("p (t r) -> p r t", r=KS)          # [128,34,6]
            pp3 = ppb[:].rearrange("p (c t) -> p c t", c=3)           # [128,3,6]
            tp3 = tpb[:].rearrange("p (c t) -> p c t", c=3)
            n2pp = feat.tile([128, 3 * TAB], F32)
            nc.vector.tensor_scalar_mul(n2pp[:], ppb[:], -2.0)
            n2pp3 = n2pp[:].rearrange("p (c t) -> p c t", c=3)

            Zpp = Z4[:, 0:9, :].rearrange("p (i j) t -> p i j t", i=3)
            Ztp = Z4[:, 9:18, :].rearrange("p (i j) t -> p i j t", i=3)
            Zx = Z4[:, 18:27, :].rearrange("p (i j) t -> p i j t", i=3)
            nc.vector.tensor_mul(Zpp, _bc(pp3, 2, 3), _bc(pp3, 1, 3))
            nc.vector.tensor_mul(Ztp, _bc(tp3, 2, 3), _bc(tp3, 1, 3))
            nc.vector.tensor_mul(Zx, _bc(n2pp3, 2, 3), _bc(tp3, 1, 3))
            nc.vector.tensor_copy(Z4[:, 27:30, :], pp3)
            nc.vector.tensor_copy(Z4[:, 30:33, :], tp3)
            nc.vector.memset(Z4[:, 33, :], 1.0)

            # mask-derived per-partition vectors (fp32)
            scale_v = consts.tile([128, TAB], F32)   # m^2
            bias_v = consts.tile([128, TAB], F32)    # m^2 * eps_eff
            thr_v = consts.tile([128, TAB], F32)     # 10 * m
            nc.vector.tensor_mul(scale_v[:], ams[:], ams[:])
            nc.vector.tensor_scalar_mul(bias_v[:], scale_v[:], EPS_EFF)
            nc.vector.tensor_scalar_mul(thr_v[:], ams[:], CLAMP)

            # ------------- transposes (PE, 128-col groups = 2 blocks) ---
            Wslab_b = feat.tile([128, KS * TFB], BF16)
            Zslab_b = feat.tile([128, KS * TAB], BF16)
            nc.vector.tensor_copy(Zslab_b[:], Zslab[:])
            NGW = KS * TFB // 128   # 12 groups
            NGZ = KS * TAB // 128   # 3 groups
            WT = consts.tile([KS, NF], BF16)
            ZT = consts.tile([KS, NAPAD], BF16)
            WT5 = WT[:].rearrange("q (g s c) -> q g s c", g=NGW, s=2)
            ZT5 = ZT[:].rearrange("q (g s c) -> q g s c", g=NGZ, s=2)

            def w_transpose_half(half):
                # cast this half of the slab, then 6 groups via PE
                HC = KS * TFB // 2
                nc.vector.tensor_copy(
                    Wslab_b[:, HC * half:HC * (half + 1)],
                    Wslab[:, HC * half:HC * (half + 1)])
                for q in range(3):
                    pst = psum_pool.tile([128, 512], BF16, tag="tp")
                    for u in range(2):
                        g = 6 * half + 2 * q + u
                        nc.tensor.transpose(
                            pst[:, 128 * u:128 * (u + 1)],
                            Wslab_b[:, 128 * g:128 * (g + 1)],
                            identity[:])
                    pst3 = pst[:, 0:256].rearrange("q (u c) -> q u c", c=128)
                    for s in range(2):
                        nc.vector.tensor_copy(
                            WT5[:, 6 * half + 2 * q:6 * half + 2 * q + 2, s, :],
                            pst3[64 * s:64 * (s + 1), :, :])

            pstz = psum_pool.tile([128, 512], BF16, tag="tp")
            for g in range(NGZ):
                nc.tensor.transpose(
                    pstz[:, 128 * g:128 * (g + 1)],
                    Zslab_b[:, 128 * g:128 * (g + 1)],
                    identity[:])
            pstz3 = pstz[:, 0:128 * NGZ].rearrange("q (u c) -> q u c", c=128)
            for s in range(2):
                nc.vector.tensor_copy(
                    ZT5[:, :, s, :], pstz3[64 * s:64 * (s + 1), :, :])


            frame_feats(0, TFB // 2)
            w_transpose_half(0)
            frame_feats(TFB // 2, TFB)
            w_transpose_half(1)

            # ------------- main loop ------------------------------------
            colacc = consts.tile([128, TAB * NCH], F32)
            scratch = consts.tile([128, CH], BF16)
            ones_b = consts.tile([128, 128], BF16)
            nc.vector.memset(ones_b[:], 1.0)

            pe_chunks = []
            idx = 0
            first_red = [True]
            for ch in range(NCH):
                for a in range(TAB):
                    zt = ZT[:, 128 * a:128 * (a + 1)]
                    ps = psum_pool.tile([128, CH], F32, tag="main")
                    for m in range(CH // MMN):
                        col = ch * CH + m * MMN
                        nc.tensor.matmul(
                            ps[:, m * MMN:(m + 1) * MMN],
                            zt,
                            WT[:, col:col + MMN],
                        )
                    s = sbuf_s.tile([128, CH], BF16)
                    nc.scalar.activation(
                        out=s[:],
                        in_=ps[:],
                        func=ACTF.Sqrt,
                        bias=bias_v[:, a:a + 1],
                        scale=scale_v[:, a:a + 1],
                    )
                    if (idx * X_FUSED) % 12 < X_FUSED and X_FUSED > 0:
                        nc.vector.tensor_scalar(
                            out=scratch[:],
                            in0=s[:],
                            scalar1=thr_v[:, a:a + 1],
                            scalar2=None,
                            op0=OP.min,
                            op1=OP.add,
                            accum_out=colacc[:, idx:idx + 1],
                        )
                    else:
                        # min at 4x on DVE (also filters sqrt-NaNs), then
                        # ScalarE sums the clamped tile via Copy+accum_out.
                        d = sbuf_s.tile([128, CH], BF16, tag="dmin")
                        nc.vector.tensor_scalar(
                            out=d[:],
                            in0=s[:],
                            scalar1=thr_v[:, a:a + 1],
                            scalar2=None,
                            op0=OP.min,
                        )
                        nc.scalar.activation(
                            out=scratch[:],
                            in_=d[:],
                            func=ACTF.Copy,
                            accum_out=colacc[:, idx:idx + 1],
                        )
                    idx += 1

            # ------------- epilogue -------------------------------------
            Sc = consts.tile([128, 1], F32)
            Sc2 = consts.tile([128, 1], F32)
            Mc = consts.tile([128, 1], F32)
            nc.vector.reduce_sum(out=Sc[:], in_=colacc[:], axis=AX.X)
            nc.vector.reduce_sum(out=Mc[:], in_=amf[:], axis=AX.X)
            ones_f = consts.tile([128, 1], F32)
            nc.vector.memset(ones_f[:], 1.0)
            psfin = psum_pool.tile([1, 2], F32, tag="tp")
            nc.tensor.matmul(psfin[:, 0:1], Sc[:], ones_f[:])
            nc.tensor.matmul(psfin[:, 1:2], Mc[:], ones_f[:])
            t0 = consts.tile([1, 1], F32)
            t1 = consts.tile([1, 1], F32)
            res = consts.tile([1, 2], F32)
            nc.vector.tensor_scalar(
                out=t0[:], in0=psfin[0:1, 1:2], scalar1=EPS, scalar2=None, op0=OP.add
            )
            nc.vector.reciprocal(t1[:], t0[:])
            nc.vector.tensor_scalar(
                out=res[:, 0:1], in0=psfin[0:1, 0:1], scalar1=t1[0:1, 0:1],
                scalar2=CNORM, op0=OP.mult, op1=OP.mult,
            )
            nc.vector.tensor_copy(res[:, 1:2], t0[:])
            nc.sync.dma_start(out=out_d[:], in_=res[:])

    nc.compile()
    return nc


def prep_in_maps(inputs):
    """Full (unsharded) numpy inputs -> per-core input dicts.

    Component-major SBUF layouts: frame f = 24*p + t lives at partition p,
    block t; a [*, C]-component tensor becomes [128, C*TFB] with column
    c*TFB + t. Atoms: a = 6*p + t, padded 672 -> 768 with zeros.
    """
    f32 = np.float32

    def fr(x, comps):
        return np.ascontiguousarray(
            np.asarray(x, f32).reshape(128, TFB, comps).transpose(0, 2, 1)
        ).reshape(128, comps * TFB)

    def at(x, comps, c):
        buf = np.zeros((NAPAD, comps), f32)
        buf[:NAS] = np.asarray(x, f32).reshape(NA, comps)[c * NAS:(c + 1) * NAS]
        return np.ascontiguousarray(
            buf.reshape(128, TAB, comps).transpose(0, 2, 1)
        ).reshape(128, comps * TAB)

    pR = fr(inputs["predicted_frames_R"], 9)
    tR = fr(inputs["true_frames_R"], 9)
    pt = fr(inputs["predicted_frames_t"], 3)
    tt = fr(inputs["true_frames_t"], 3)
    am_flat = np.asarray(inputs["atom_mask"], f32).reshape(NA)
    amf = np.ascontiguousarray(am_flat).reshape(128, NA // 128)

    in_maps = []
    for c in range(NCORES):
        amp = np.zeros((NAPAD,), f32)
        amp[:NAS] = am_flat[c * NAS:(c + 1) * NAS]
        in_maps.append({
            "pr": pR, "tr": tR, "pt": pt, "tt": tt,
            "ident": np.eye(128, dtype=f32),
            "pp": at(inputs["predicted_atom_positions"], 3, c),
            "tp": at(inputs["true_atom_positions"], 3, c),
            "am": np.ascontiguousarray(amp.reshape(128, TAB)),
            "amf": amf,
        })
    return in_maps


_NC_CACHE = None


def _get_nc():
    global _NC_CACHE
    if _NC_CACHE is None:
        _NC_CACHE = build_nc()
    return _NC_CACHE


def kernel(**inputs):
    nc = _get_nc()
    in_maps = prep_in_maps(inputs)
    r = run_bass_kernel_spmd(nc, in_maps, core_ids=list(range(NCORES)))
    total = np.float32(0.0)
    for i in range(NCORES):
        total += np.float32(r.results[i]["out"][0, 0])
    return np.array([total], dtype=np.float32)


# revision 39
# speedup vs baseline: 1.2057x; 1.2057x over previous
"""AllAtomFAPE loss kernel for Trainium2 (8 NeuronCores, SPMD).

Problem: b=1, N=384 res, F=8 frames/res -> NF=3072 frames; A=14 atoms/res
-> NA=5376 atoms. Output: scalar (shape (1,)) masked clamped FAPE.

Algorithm (factorized pairwise distance):
  With P = pR pR^T, T = tR tR^T, M = pR tR^T (per frame, 3x3),
    d2(f,a) = (pp-pt)^T P (pp-pt) + (tp-tt)^T T (tp-tt)
              - 2 (pp-pt)^T M (tp-tt)
  expands into a K=34 dot product between a frame feature vector W[:,f]
  and an atom feature vector Z[:,a]:
    rows 0-8   : P[i,j]            <->  pp_i pp_j
    rows 9-17  : T[i,j]            <->  tp_i tp_j
    rows 18-26 : M[i,j]            <->  -2 pp_i tp_j
    rows 27-29 : 2(M tt - P pt)    <->  pp
    rows 30-32 : 2(M^T pt - T tt)  <->  tp
    row  33    : c_f               <->  1
  so the whole pairwise computation is one (34 x NF) x (34 x NA) matmul
  on the TensorEngine (bf16). Then (ScalarE) d = m_a*sqrt(d2+eps) via
  sqrt(scale*x+bias) with per-partition scale=m^2, bias=m^2*eps, and
  (VectorE) fused clamp+reduce: tensor_scalar(min thr=10*m_a, accum add).

Sharding: atoms sharded across the 8 cores (672 each, padded to 768);
frames replicated. Each core emits one partial scalar; the host sums
the 8 partials (the gather/unshard step).

Layouts: frame f = 24*p + t (partition p, block t); slabs are
row-major in the feature index r with the block index t innermost
(unit stride) so DVE ops hit the packed 2x bf16 mode. Feature slabs
are transposed to [34, entity] via PE transposes (3 blocks packed per
transpose -> [102, 128] in PSUM) + strided DVE copies.
"""

import numpy as np

import concourse.bacc as bacc
import concourse.bass as bass
import concourse.tile as tile
from concourse import mybir
from concourse.bass_utils import run_bass_kernel_spmd

F32 = mybir.dt.float32
BF16 = mybir.dt.bfloat16
AX = mybir.AxisListType
OP = mybir.AluOpType
ACTF = mybir.ActivationFunctionType

NCORES = 8
NF = 3072          # frames (N*F)
TFB = 24           # frame blocks per partition (f = 24*p + t)
NA = 5376          # atoms (N*A)
NAS = NA // NCORES  # 672 atoms per core
NAPAD = 768        # padded per-core atoms
TAB = 6            # atom blocks per partition (a = 6*p + t)
K = 34             # feature dim
KS = 64            # stored feature rows (zero-padded, 128/KS-aligned for
                   # the DMA xbar transpose + 32-aligned strip copies)
CH = 1536          # frame chunk (PSUM cols) per ACT/DVE op
NCH = NF // CH     # 2 chunks
MMN = 512          # matmul moving free dim
X_FUSED = 12       # chunks (of 12) whose clamp+reduce is fused on DVE at 1x
                   # (measured cheapest reduction: ACT Copy+accum ~2.4us/chunk
                   # and PE ones-matmuls both regressed).
EPS = 1e-4
EPS_EFF = EPS      # sqrt(neg)=NaN is filtered by the DVE min (min(NaN,thr)=thr,
                   # verified on HW); rare tiny-d2 pairs hitting that path
                   # contribute ~1e-4 relative error.
CLAMP = 10.0
ZSCALE = 10.0
CNORM = float(1.0 / (ZSCALE * (3072.0 + EPS)))


def _bc(ap, dim, n):
    """Broadcast AP along a new axis at position `dim` (stride-0), n copies."""
    return ap.unsqueeze(dim).to_broadcast(
        tuple(ap.shape[:dim]) + (n,) + tuple(ap.shape[dim:])
    )


def build_nc():
    nc = bacc.Bacc(None)

    # inputs consolidated into two params -> two DMAs (queue latency is
    # ~2us per dma_start; nine separate loads stalled the feature ops)
    FRW = 2 * 9 * TFB + 2 * 3 * TFB          # pr|tr|pt|tt = 576
    ATW = 2 * 3 * TAB + TAB + NA // 128 + 128  # pp|tp|am|amf|ident = 212
    fr_d = nc.declare_dram_parameter("fr", [128, FRW], F32, isOutput=False)
    at_d = nc.declare_dram_parameter("at", [128, ATW], F32, isOutput=False)
    out_d = nc.declare_dram_parameter("out", [1, 2], F32, isOutput=True)

    with tile.TileContext(nc) as tc:
        with (
            tc.tile_pool(name="consts", bufs=1) as consts,
            tc.tile_pool(name="feat", bufs=1) as feat,
            tc.tile_pool(name="psum", bufs=2, space="PSUM") as psum_pool,
            tc.tile_pool(name="sbuf_s", bufs=3) as sbuf_s,
        ):
            # ---------------- input DMAs (two consolidated loads) --------
            frs = consts.tile([128, FRW], F32)
            ats = consts.tile([128, ATW], F32)
            nc.sync.dma_start(out=frs[:], in_=fr_d[:])
            nc.scalar.dma_start(out=ats[:], in_=at_d[:])
            fr_ap = frs[:]
            pRs = fr_ap[:, 0:216]
            tRs = fr_ap[:, 216:432]
            pts = fr_ap[:, 432:504]
            tts = fr_ap[:, 504:576]
            at_ap = ats[:]
            pps = at_ap[:, 0:18]
            tps = at_ap[:, 18:36]
            ams = at_ap[:, 36:42]
            amf = at_ap[:, 42:84]
            identf = at_ap[:, 84:212]
            identity = consts.tile([128, 128], BF16)
            nc.vector.tensor_copy(identity[:], identf)
            pRb, tRb, ptb, ttb, ppb, tpb = pRs, tRs, pts, tts, pps, tps

            # ------------- frame features (fp32, two t-halves) ----------
            # Slab col = KS*t + r. Computed per t-half so the first half's
            # transposes + main-loop chunk ch=0 overlap the second half.
            Wslab = feat.tile([128, KS * TFB], F32)
            nc.vector.memset(Wslab[:], 0.0)
            TH = TFB // 2
            mul0 = feat.tile([128, 9 * TH], F32)
            mul1 = feat.tile([128, 9 * TH], F32)
            mul2 = feat.tile([128, 9 * TH], F32)
            tmp3b = feat.tile([128, 3 * TH], F32)
            tmp3 = feat.tile([128, 3 * TFB], F32)
            tmp1 = feat.tile([128, TFB], F32)
            Ppt = feat.tile([128, 3 * TFB], F32)
            Mtt = feat.tile([128, 3 * TFB], F32)
            Ttt = feat.tile([128, 3 * TFB], F32)
            Mtp = feat.tile([128, 3 * TFB], F32)

            W4a = Wslab[:].rearrange("p (t r) -> p r t", r=KS)         # [128,64,24]
            R4a = pRb.rearrange("p (c t) -> p c t", c=9)
            T4a = tRb.rearrange("p (c t) -> p c t", c=9)
            pt3a = ptb.rearrange("p (c t) -> p c t", c=3)           # [128,3,24]
            tt3a = ttb.rearrange("p (c t) -> p c t", c=3)
            m0v = mul0[:].rearrange("p (i j t) -> p i j t", i=3, j=3)  # contig
            m1v = mul1[:].rearrange("p (i j t) -> p i j t", i=3, j=3)
            m2v = mul2[:].rearrange("p (i j t) -> p i j t", i=3, j=3)
            t3bv = tmp3b[:].rearrange("p (c t) -> p c t", c=3)

            def frame_feats(lo, hi):
                W4 = W4a[:, :, lo:hi]
                R4 = R4a[:, :, lo:hi].rearrange("p (i k) t -> p i k t", i=3)
                T4 = T4a[:, :, lo:hi].rearrange("p (i k) t -> p i k t", i=3)
                pt3 = pt3a[:, :, lo:hi]
                tt3 = tt3a[:, :, lo:hi]

                def gram(out4, A4, B4):
                    # 3 muls to contiguous temps, adds; only the last add
                    # writes the (KS-strided) slab rows.
                    a = lambda k: _bc(A4[:, :, k, :], 2, 3)
                    b = lambda k: _bc(B4[:, :, k, :], 1, 3)
                    nc.vector.tensor_mul(m0v, a(0), b(0))
                    nc.vector.tensor_mul(m1v, a(1), b(1))
                    nc.vector.tensor_mul(m2v, a(2), b(2))
                    nc.vector.tensor_add(m0v, m0v, m1v)
                    nc.vector.tensor_add(out4, m0v, m2v)

                Pv = W4[:, 0:9, :].rearrange("p (i j) t -> p i j t", i=3)
                Tv = W4[:, 9:18, :].rearrange("p (i j) t -> p i j t", i=3)
                Mv = W4[:, 18:27, :].rearrange("p (i j) t -> p i j t", i=3)
                gram(Pv, R4, R4)
                gram(Tv, T4, T4)
                gram(Mv, R4, T4)

                mjit = mul0[:].rearrange("p (j i t) -> p j i t", j=3, i=3)

                def matvec(out3, Q, vec3, transpose=False):
                    # one wide mul over (j,i,t), then two adds
                    qv = Q.transpose([0, 2, 1, 3]) if not transpose else Q
                    mj = mjit[:, :, :, 0:vec3.shape[2]]
                    nc.vector.tensor_mul(mj, qv, _bc(vec3, 2, 3))
                    nc.vector.tensor_add(t3bv, mj[:, 0, :, :], mj[:, 1, :, :])
                    nc.vector.tensor_add(out3, t3bv, mj[:, 2, :, :])

                Ppt3 = Ppt[:].rearrange("p (c t) -> p c t", c=3)[:, :, lo:hi]
                Mtt3 = Mtt[:].rearrange("p (c t) -> p c t", c=3)[:, :, lo:hi]
                Ttt3 = Ttt[:].rearrange("p (c t) -> p c t", c=3)[:, :, lo:hi]
                Mtp3 = Mtp[:].rearrange("p (c t) -> p c t", c=3)[:, :, lo:hi]
                matvec(Ppt3, Pv, pt3)
                matvec(Mtt3, Mv, tt3)
                matvec(Ttt3, Tv, tt3)
                matvec(Mtp3, Mv, pt3, transpose=True)  # M^T pt

                tmp3v = tmp3[:].rearrange("p (c t) -> p c t", c=3)[:, :, lo:hi]
                nc.vector.tensor_sub(tmp3v, Mtt3, Ppt3)
                nc.vector.tensor_scalar_mul(W4[:, 27:30, :], tmp3v, 2.0)
                nc.vector.tensor_sub(tmp3v, Mtp3, Ttt3)
                nc.vector.tensor_scalar_mul(W4[:, 30:33, :], tmp3v, 2.0)

                # cf row 33: pt.(Ppt - 2*Mtt) + tt.Ttt
                cfb = W4[:, 33, :]
                t1b = tmp1[:, lo:hi]
                nc.vector.tensor_sub(tmp3v, Ppt3, Mtt3)
                nc.vector.tensor_sub(tmp3v, tmp3v, Mtt3)
                # dot products via one wide mul each, then pairwise adds
                pd = t3bv  # [128,3,TH]
                nc.vector.tensor_mul(pd, tmp3v, pt3)
                nc.vector.tensor_add(t1b, pd[:, 0, :], pd[:, 1, :])
                nc.vector.tensor_add(cfb, t1b, pd[:, 2, :])
                nc.vector.tensor_mul(pd, Ttt3, tt3)
                nc.vector.tensor_add(t1b, pd[:, 0, :], pd[:, 1, :])
                nc.vector.tensor_add(t1b, t1b, pd[:, 2, :])
                nc.vector.tensor_add(cfb, cfb, t1b)

            # ------------- atom features (sharded) ----------------------
            Zslab = feat.tile([128, KS * TAB], F32)
            nc.vector.memset(Zslab[:], 0.0)
            Z4 = Zslab[:].rearrange("p (t r) -> p r t", r=KS)          # [128,34,6]
            pp3 = ppb.rearrange("p (c t) -> p c t", c=3)           # [128,3,6]
            tp3 = tpb.rearrange("p (c t) -> p c t", c=3)
            n2pp = feat.tile([128, 3 * TAB], F32)
            nc.vector.tensor_scalar_mul(n2pp[:], ppb, -2.0)
            n2pp3 = n2pp[:].rearrange("p (c t) -> p c t", c=3)

            Zpp = Z4[:, 0:9, :].rearrange("p (i j) t -> p i j t", i=3)
            Ztp = Z4[:, 9:18, :].rearrange("p (i j) t -> p i j t", i=3)
            Zx = Z4[:, 18:27, :].rearrange("p (i j) t -> p i j t", i=3)
            nc.vector.tensor_mul(Zpp, _bc(pp3, 2, 3), _bc(pp3, 1, 3))
            nc.vector.tensor_mul(Ztp, _bc(tp3, 2, 3), _bc(tp3, 1, 3))
            nc.vector.tensor_mul(Zx, _bc(n2pp3, 2, 3), _bc(tp3, 1, 3))
            nc.vector.tensor_copy(Z4[:, 27:30, :], pp3)
            nc.vector.tensor_copy(Z4[:, 30:33, :], tp3)
            nc.vector.memset(Z4[:, 33, :], 1.0)

            # mask-derived per-partition vectors (fp32)
            scale_v = consts.tile([128, TAB], F32)   # m^2
            bias_v = consts.tile([128, TAB], F32)    # m^2 * eps_eff
            thr_v = consts.tile([128, TAB], F32)     # 10 * m
            nc.vector.tensor_mul(scale_v[:], ams, ams)
            nc.vector.tensor_scalar_mul(bias_v[:], scale_v[:], EPS_EFF)
            nc.vector.tensor_scalar_mul(thr_v[:], ams, CLAMP)

            # ------------- transposes (PE, 128-col groups = 2 blocks) ---
            Wslab_b = feat.tile([128, KS * TFB], BF16)
            Zslab_b = feat.tile([128, KS * TAB], BF16)
            nc.vector.tensor_copy(Zslab_b[:], Zslab[:])
            NGW = KS * TFB // 128   # 12 groups
            NGZ = KS * TAB // 128   # 3 groups
            WT = consts.tile([KS, NF], BF16)
            ZT = consts.tile([KS, NAPAD], BF16)
            WT5 = WT[:].rearrange("q (g s c) -> q g s c", g=NGW, s=2)
            ZT5 = ZT[:].rearrange("q (g s c) -> q g s c", g=NGZ, s=2)

            def w_transpose_half(half):
                # cast this half of the slab, then 6 groups via PE
                HC = KS * TFB // 2
                nc.vector.tensor_copy(
                    Wslab_b[:, HC * half:HC * (half + 1)],
                    Wslab[:, HC * half:HC * (half + 1)])
                for q in range(3):
                    pst = psum_pool.tile([128, 512], BF16, tag="tp")
                    for u in range(2):
                        g = 6 * half + 2 * q + u
                        nc.tensor.transpose(
                            pst[:, 128 * u:128 * (u + 1)],
                            Wslab_b[:, 128 * g:128 * (g + 1)],
                            identity[:])
                    pst3 = pst[:, 0:256].rearrange("q (u c) -> q u c", c=128)
                    for s in range(2):
                        nc.vector.tensor_copy(
                            WT5[:, 6 * half + 2 * q:6 * half + 2 * q + 2, s, :],
                            pst3[64 * s:64 * (s + 1), :, :])

            pstz = psum_pool.tile([128, 512], BF16, tag="tp")
            for g in range(NGZ):
                nc.tensor.transpose(
                    pstz[:, 128 * g:128 * (g + 1)],
                    Zslab_b[:, 128 * g:128 * (g + 1)],
                    identity[:])
            pstz3 = pstz[:, 0:128 * NGZ].rearrange("q (u c) -> q u c", c=128)
            for s in range(2):
                nc.vector.tensor_copy(
                    ZT5[:, :, s, :], pstz3[64 * s:64 * (s + 1), :, :])


            frame_feats(0, TFB // 2)
            w_transpose_half(0)
            frame_feats(TFB // 2, TFB)
            w_transpose_half(1)

            # ------------- main loop ------------------------------------
            colacc = consts.tile([128, TAB * NCH], F32)
            scratch = consts.tile([128, CH], BF16)
            ones_b = consts.tile([128, 128], BF16)
            nc.vector.memset(ones_b[:], 1.0)

            pe_chunks = []
            idx = 0
            first_red = [True]
            for ch in range(NCH):
                for a in range(TAB):
                    zt = ZT[:, 128 * a:128 * (a + 1)]
                    ps = psum_pool.tile([128, CH], F32, tag="main")
                    for m in range(CH // MMN):
                        col = ch * CH + m * MMN
                        nc.tensor.matmul(
                            ps[:, m * MMN:(m + 1) * MMN],
                            zt,
                            WT[:, col:col + MMN],
                        )
                    s = sbuf_s.tile([128, CH], BF16)
                    nc.scalar.activation(
                        out=s[:],
                        in_=ps[:],
                        func=ACTF.Sqrt,
                        bias=bias_v[:, a:a + 1],
                        scale=scale_v[:, a:a + 1],
                    )
                    if (idx * X_FUSED) % 12 < X_FUSED and X_FUSED > 0:
                        nc.vector.tensor_scalar(
                            out=scratch[:],
                            in0=s[:],
                            scalar1=thr_v[:, a:a + 1],
                            scalar2=None,
                            op0=OP.min,
                            op1=OP.add,
                            accum_out=colacc[:, idx:idx + 1],
                        )
                    else:
                        # min at 4x on DVE (also filters sqrt-NaNs), then
                        # ScalarE sums the clamped tile via Copy+accum_out.
                        d = sbuf_s.tile([128, CH], BF16, tag="dmin")
                        nc.vector.tensor_scalar(
                            out=d[:],
                            in0=s[:],
                            scalar1=thr_v[:, a:a + 1],
                            scalar2=None,
                            op0=OP.min,
                        )
                        nc.scalar.activation(
                            out=scratch[:],
                            in_=d[:],
                            func=ACTF.Copy,
                            accum_out=colacc[:, idx:idx + 1],
                        )
                    idx += 1

            # ------------- epilogue -------------------------------------
            Sc = consts.tile([128, 1], F32)
            Sc2 = consts.tile([128, 1], F32)
            Mc = consts.tile([128, 1], F32)
            nc.vector.reduce_sum(out=Sc[:], in_=colacc[:], axis=AX.X)
            nc.vector.reduce_sum(out=Mc[:], in_=amf, axis=AX.X)
            ones_f = consts.tile([128, 1], F32)
            nc.vector.memset(ones_f[:], 1.0)
            psfin = psum_pool.tile([1, 2], F32, tag="tp")
            nc.tensor.matmul(psfin[:, 0:1], Sc[:], ones_f[:])
            nc.tensor.matmul(psfin[:, 1:2], Mc[:], ones_f[:])
            t0 = consts.tile([1, 1], F32)
            t1 = consts.tile([1, 1], F32)
            res = consts.tile([1, 2], F32)
            nc.vector.tensor_scalar(
                out=t0[:], in0=psfin[0:1, 1:2], scalar1=EPS, scalar2=None, op0=OP.add
            )
            nc.vector.reciprocal(t1[:], t0[:])
            nc.vector.tensor_scalar(
                out=res[:, 0:1], in0=psfin[0:1, 0:1], scalar1=t1[0:1, 0:1],
                scalar2=CNORM, op0=OP.mult, op1=OP.mult,
            )
            nc.vector.tensor_copy(res[:, 1:2], t0[:])
            nc.sync.dma_start(out=out_d[:], in_=res[:])

    nc.compile()
    return nc


def prep_in_maps(inputs):
    """Full (unsharded) numpy inputs -> per-core input dicts.

    Component-major SBUF layouts: frame f = 24*p + t lives at partition p,
    block t; a [*, C]-component tensor becomes [128, C*TFB] with column
    c*TFB + t. Atoms: a = 6*p + t, padded 672 -> 768 with zeros.
    """
    f32 = np.float32

    def fr(x, comps):
        return np.ascontiguousarray(
            np.asarray(x, f32).reshape(128, TFB, comps).transpose(0, 2, 1)
        ).reshape(128, comps * TFB)

    def at(x, comps, c):
        buf = np.zeros((NAPAD, comps), f32)
        buf[:NAS] = np.asarray(x, f32).reshape(NA, comps)[c * NAS:(c + 1) * NAS]
        return np.ascontiguousarray(
            buf.reshape(128, TAB, comps).transpose(0, 2, 1)
        ).reshape(128, comps * TAB)

    pR = fr(inputs["predicted_frames_R"], 9)
    tR = fr(inputs["true_frames_R"], 9)
    pt = fr(inputs["predicted_frames_t"], 3)
    tt = fr(inputs["true_frames_t"], 3)
    am_flat = np.asarray(inputs["atom_mask"], f32).reshape(NA)
    amf = np.ascontiguousarray(am_flat).reshape(128, NA // 128)

    fr = np.ascontiguousarray(np.concatenate([pR, tR, pt, tt], axis=1))
    ident = np.eye(128, dtype=f32)
    in_maps = []
    for c in range(NCORES):
        amp = np.zeros((NAPAD,), f32)
        amp[:NAS] = am_flat[c * NAS:(c + 1) * NAS]
        atc = np.ascontiguousarray(np.concatenate([
            at(inputs["predicted_atom_positions"], 3, c),
            at(inputs["true_atom_positions"], 3, c),
            amp.reshape(128, TAB),
            amf,
            ident,
        ], axis=1))
        in_maps.append({"fr": fr, "at": atc})
    return in_maps


_NC_CACHE = None


def _get_nc():
    global _NC_CACHE
    if _NC_CACHE is None:
        _NC_CACHE = build_nc()
    return _NC_CACHE


def kernel(**inputs):
    nc = _get_nc()
    in_maps = prep_in_maps(inputs)
    r = run_bass_kernel_spmd(nc, in_maps, core_ids=list(range(NCORES)))
    total = np.float32(0.0)
    for i in range(NCORES):
        total += np.float32(r.results[i]["out"][0, 0])
    return np.array([total], dtype=np.float32)
